# revision 26
# baseline (speedup 1.0000x reference)
"""Trainium2 Bass kernel for nn_Attention_84756884619871.

Causal multi-head attention (B=2, S=2048, D=2048, H=16, Dh=128) with RoPE,
fused QKV projection and output projection.

Sharding (8 NeuronCores): tensor-parallel over heads, TP=8 (2 heads/core),
each core processes BOTH batches.  Per core:
  - Phase A: single pass over x^T (both batches, 4 s-chunks of 512): fused
    q^T,k^T (RoPE applied, k pre-scaled by 1/sqrt(dh)) and v projection.
  - Phase B: flash-style attention in score-transposed space p^T[t,s] with a
    2-deep software pipeline (scores for j+2 issued before exp/ctx of j), so
    the PE never waits on the scalar-engine exp.  Softmax denominator via
    ones-vector matmul; no max-subtraction (scores are small; fp32 exp safe).
  - The output projection is sharded over SEQUENCE: core c owns s rows
    [256c, 256c+256) of each half (batch g=c//4, slice tp=c%4).  After each
    s-half's attention, one 8-way AllToAll redistributes ctx^T so every core
    has all 16 heads for its own s-rows (wire: 0.875 MB/core vs 3 MB for the
    head-AllGather alternative).  A2A#0 overlaps second-half attention;
    A2A#1 overlaps the first half's output projection.
  - Phase C: out[s_mine, 0:2048] = ctx_full^T.T @ w_o, K-accumulated over all
    16 head-blocks. All matmuls bf16 (full PE rate).
Host assembles the full [2,2048,2048] output from per-core s-row slices.
"""

import numpy as np
import ml_dtypes

import concourse.bass as bass
import concourse.tile as tile
import concourse.mybir as mybir
from concourse import bacc
from contextlib import ExitStack

F32 = mybir.dt.float32
BF = mybir.dt.bfloat16
AF = mybir.ActivationFunctionType

D = 2048
S = 2048
NCORES = 8
HLOC = 2           # heads per core (TP=8 over 16 heads)
DH = 128
SCALE = float(1.0 / np.sqrt(DH))

# phase-A s-chunks per batch (first chunk small so the first matmul starts
# early).  The host pre-swizzles x to match this chunking exactly.
CH0 = [(0, 384), (384, 896), (896, 1408), (1408, 1920), (1920, 2048)]
CH1 = [(0, 512), (512, 1024), (1024, 1536), (1536, 2048)]
# wqkv column blocks (q head0 | q head1 | k | v); host swizzles to match
WBLK = [(0, 128), (128, 256), (256, 512), (512, 768)]

_STATE: dict = {}


def _chunks(sb, j):
    """Score columns [c0, 1024) for key-block j in s-half sb, <=512 pieces."""
    dj = j - 8 * sb
    c0 = max(0, 128 * dj)
    if c0 < 512:
        return [(c0, 512), (512, 1024)]
    return [(c0, 1024)]


def _build():
    nc = bacc.Bacc("TRN2", target_bir_lowering=False, debug=False,
                   num_devices=NCORES)
    # x / wqkv / wo are host-pre-swizzled so every DMA reads large contiguous
    # per-partition runs (the naive (ko ki)-interleaved APs fragment into
    # <=1KB descriptors and run at ~1/3 of HBM rate).
    xT0 = nc.dram_tensor("xT0", [128, 16 * S], BF, kind="ExternalInput")
    xT1 = nc.dram_tensor("xT1", [128, 16 * S], BF, kind="ExternalInput")
    wqkv = nc.dram_tensor("wqkv", [128, 16 * 768], BF, kind="ExternalInput")
    wo = nc.dram_tensor("wo", [128, 16 * D], BF, kind="ExternalInput")
    cosq = nc.dram_tensor("cosq", [128, S], BF, kind="ExternalInput")
    sinq = nc.dram_tensor("sinq", [128, S], BF, kind="ExternalInput")
    cosk = nc.dram_tensor("cosk", [128, S], BF, kind="ExternalInput")
    sink = nc.dram_tensor("sink", [128, S], BF, kind="ExternalInput")
    trimask = nc.dram_tensor("trimask", [128, 128], BF, kind="ExternalInput")
    out = nc.dram_tensor("out", [512, D], F32, kind="ExternalOutput")

    def _x_src(b, c):
        """2D source AP for chunk c of batch b in the swizzled x layout."""
        chunks = CH0 if b == 0 else CH1
        xT = xT0 if b == 0 else xT1
        off = 0
        for (s0, s1) in chunks:
            if (s0, s1) == c:
                return xT.ap()[:, off:off + 16 * (s1 - s0)]
            off += 16 * (s1 - s0)
        raise ValueError(c)

    def _w_src(c0, c1):
        """2D source AP for wqkv col block [c0, c1) in the swizzled layout."""
        off = 0
        for (b0, b1) in WBLK:
            if (b0, b1) == (c0, c1):
                return wqkv.ap()[:, 16 * off:16 * off + 16 * (c1 - c0)]
            off += b1 - b0
        raise ValueError((c0, c1))

    with tile.TileContext(nc) as tc, ExitStack() as top:
        # ---- persistent small tensors -------------------------------------
        per = top.enter_context(tc.tile_pool(name="persist", bufs=1))
        mask_sb = per.tile([128, 128], BF, name="mask")
        ones_c0 = per.tile([128, 1], F32, name="ones_c0")
        nc.vector.memset(ones_c0[:], 1.0)
        ones_col = per.tile([128, 1], BF, name="ones_col")
        nc.vector.tensor_copy(ones_col[:], ones_c0[:])

        dram = top.enter_context(tc.tile_pool(name="dram", bufs=1, space="DRAM"))
        a2a_in = [dram.tile([2048, 256], BF, name=f"a2ain{sb}") for sb in range(2)]
        a2a_out = [dram.tile([2048, 256], BF, name=f"a2aout{sb}") for sb in range(2)]

        # persistent q/k (rotated, transposed) and v for both batches
        qk_pool = top.enter_context(tc.tile_pool(name="qkpool", bufs=1))
        qrot = [[qk_pool.tile([128, S], BF, name=f"qrot{b}_{h}")
                 for h in range(HLOC)] for b in range(2)]
        krot = [[qk_pool.tile([128, S], BF, name=f"krot{b}_{h}")
                 for h in range(HLOC)] for b in range(2)]
        v_pool = top.enter_context(tc.tile_pool(name="vpool", bufs=1))
        vsb = [[v_pool.tile([128, HLOC * 128], BF, name=f"v{b}_{j}")
                for j in range(16)] for b in range(2)]

        # ---- phase A: fused qkv projection + RoPE (one pass over x) -------
        with ExitStack() as st:
            wq_pool = st.enter_context(tc.tile_pool(name="wqp", bufs=1))
            # one tile per wqkv col block so each DMA is 2D-contiguous (HWDGE)
            wqk_sb = [wq_pool.tile([128, 16, b1 - b0], BF, name=f"wqkv_sb{i}")
                      for i, (b0, b1) in enumerate(WBLK)]
            cs_pool = st.enter_context(tc.tile_pool(name="csp", bufs=1))
            cosq_sb = cs_pool.tile([128, S], BF, name="cosq_sb")
            sinq_sb = cs_pool.tile([128, S], BF, name="sinq_sb")
            cosk_sb = cs_pool.tile([128, S], BF, name="cosk_sb")
            sink_sb = cs_pool.tile([128, S], BF, name="sink_sb")
            xt_pool = st.enter_context(tc.tile_pool(name="xtp", bufs=3))
            rt_pool = st.enter_context(tc.tile_pool(name="ropet", bufs=3))
            ps_qk = st.enter_context(tc.tile_pool(name="psqk", bufs=3, space="PSUM"))
            ps_v = st.enter_context(tc.tile_pool(name="psv", bufs=2, space="PSUM"))

            # stage DMAs: small first x chunk + q-head-0 weights first so the
            # first matmul starts as early as possible
            order = [(0, c) for c in CH0] + [(1, c) for c in CH1]
            xts = {}

            def load_chunk(b, c):
                s0, s1 = c
                t = xt_pool.tile([128, 16, s1 - s0], BF, tag="xt",
                                 name=f"xt{b}_{s0}")
                nc.sync.dma_start(t[:].rearrange("ki ko w -> ki (ko w)"),
                                  _x_src(b, c))
                xts[(b, c)] = t

            def load_w(i):
                b0, b1 = WBLK[i]
                nc.sync.dma_start(
                    wqk_sb[i][:].rearrange("ki ko w -> ki (ko w)"),
                    _w_src(b0, b1))

            load_chunk(*order[0])
            load_w(0)
            load_w(1)
            load_chunk(*order[1])
            nc.sync.dma_start(cosq_sb[:], cosq.ap())
            nc.sync.dma_start(sinq_sb[:], sinq.ap())
            load_w(2)
            nc.sync.dma_start(cosk_sb[:], cosk.ap())
            nc.sync.dma_start(sink_sb[:], sink.ap())
            load_w(3)
            nc.sync.dma_start(mask_sb[:], trimask.ap())

            for idx, (b, c) in enumerate(order):
                    if idx + 2 < len(order):
                        load_chunk(*order[idx + 2])

                    xt = xts.pop((b, c))
                    s0, s1 = c
                    w = s1 - s0
                    sl = slice(s0, s1)
                    for m in range(4):
                        pq = ps_qk.tile([128, 512], F32, tag="psqk",
                                        name=f"pq{b}_{s0}_{m}")
                        for ko in range(16):
                            if m < 2:
                                lhsT = wqk_sb[m][:, ko, 0:128]
                            else:
                                mm2 = 128 * (m - 2)
                                lhsT = wqk_sb[2][:, ko, mm2:mm2 + 128]
                            nc.tensor.matmul(
                                pq[:, 0:w], lhsT,
                                xt[:, ko, :], start=(ko == 0), stop=(ko == 15),
                            )
                        # RoPE: DVE reads PSUM directly (partition-offset
                        # reads are PSUM-source-only); bf16 temps/outputs.
                        if m < 2:
                            cos_t, sin_t = cosq_sb, sinq_sb
                            dest = qrot[b][m][:, sl]
                        else:
                            cos_t, sin_t = cosk_sb, sink_sb
                            dest = krot[b][m - 2][:, sl]
                        t1 = rt_pool.tile([128, 512], BF, tag="t1",
                                          name=f"t1_{b}_{s0}_{m}")
                        nc.vector.tensor_mul(t1[:, 0:w], pq[:, 0:w], cos_t[:, sl])
                        t2 = rt_pool.tile([128, 512], BF, tag="t2",
                                          name=f"t2_{b}_{s0}_{m}")
                        nc.vector.tensor_mul(t2[0:64, 0:w], pq[64:128, 0:w],
                                             sin_t[0:64, sl])
                        nc.vector.tensor_mul(t2[64:128, 0:w], pq[0:64, 0:w],
                                             sin_t[64:128, sl])
                        nc.vector.tensor_add(dest, t1[:, 0:w], t2[:, 0:w])
                    for u in range(w // 128):
                        j = s0 // 128 + u
                        pv = ps_v.tile([128, HLOC * 128], F32, tag="psv",
                                       name=f"pv{b}_{j}")
                        for ko in range(16):
                            nc.tensor.matmul(
                                pv[:], xt[:, ko, 128 * u:128 * u + 128],
                                wqk_sb[3][:, ko, :],
                                start=(ko == 0), stop=(ko == 15),
                            )
                        nc.scalar.copy(vsb[b][j][:], pv[:])

        # ---- phases B+C -----------------------------------------------------
        with ExitStack() as bc:
            wo_pool = bc.enter_context(tc.tile_pool(name="wop", bufs=1))
            wo_sb = wo_pool.tile([128, 16, D], BF, name="wo_sb")
            p_pool = bc.enter_context(tc.tile_pool(name="pp", bufs=3))
            misc = bc.enter_context(tc.tile_pool(name="miscb", bufs=2))
            cg_pool = bc.enter_context(tc.tile_pool(name="cgp", bufs=1))
            ctxg = [cg_pool.tile([128, 16, 256], BF, name=f"ctxg{half}")
                    for half in range(2)]

            # ---- phase B: attention, software-pipelined -------------------
            with ExitStack() as stb:
                sc_ps = stb.enter_context(
                    tc.tile_pool(name="scps", bufs=2, space="PSUM"))
                ctx_ps = stb.enter_context(
                    tc.tile_pool(name="ctxps", bufs=1, space="PSUM"))
                l_ps = stb.enter_context(
                    tc.tile_pool(name="lps", bufs=1, space="PSUM"))
                for sb in range(2):
                    for b in range(2):
                        for h in range(HLOC):
                            jmax = 8 * sb + 8
                            sc_tiles = {}

                            def issue_sc(j, sb=sb, b=b, h=h):
                                t = sc_ps.tile([128, 1024], F32, tag="sc",
                                               name=f"sc{sb}{b}{h}_{j}")
                                for (cs, ce) in _chunks(sb, j):
                                    nc.tensor.matmul(
                                        t[:, cs:ce],
                                        krot[b][h][:, 128 * j:128 * j + 128],
                                        qrot[b][h][:, 1024 * sb + cs:1024 * sb + ce],
                                        start=True, stop=True,
                                    )
                                sc_tiles[j] = t

                            issue_sc(0)
                            issue_sc(1)
                            ctx = ctx_ps.tile([128, 1024], F32, tag="ctx",
                                              name=f"ctx{sb}{b}{h}")
                            lps = l_ps.tile([1, 1024], F32, tag="l",
                                            name=f"l{sb}{b}{h}")
                            for j in range(jmax):
                                if j + 2 < jmax:
                                    issue_sc(j + 2)
                                sc_t = sc_tiles.pop(j)
                                cks = _chunks(sb, j)
                                dj = j - 8 * sb
                                c0 = cks[0][0]
                                p_t = p_pool.tile([128, 1024], BF, tag="p",
                                                  name=f"p{sb}{b}{h}_{j}")
                                if j < 2:
                                    # chunk-level exp at instance start so the
                                    # first ctx matmul never waits a full exp
                                    for (cs, ce) in cks:
                                        nc.scalar.activation(p_t[:, cs:ce],
                                                             sc_t[:, cs:ce],
                                                             AF.Exp)
                                else:
                                    nc.scalar.activation(p_t[:, c0:1024],
                                                         sc_t[:, c0:1024], AF.Exp)
                                if dj >= 0:
                                    dsl = slice(128 * dj, 128 * dj + 128)
                                    nc.vector.tensor_mul(p_t[:, dsl], p_t[:, dsl],
                                                         mask_sb[:])
                                last = (j == jmax - 1)
                                for (cs, ce) in cks:
                                    nc.tensor.matmul(
                                        ctx[:, cs:ce],
                                        vsb[b][j][:, 128 * h:128 * h + 128],
                                        p_t[:, cs:ce], start=(j == 0), stop=last,
                                        skip_group_check=True,
                                    )
                                for (cs, ce) in cks:
                                    nc.tensor.matmul(
                                        lps[0:1, cs:ce], ones_col[:], p_t[:, cs:ce],
                                        start=(j == 0), stop=last,
                                        skip_group_check=True,
                                    )
                            # normalize: ctxn = ctx * (1/l).  No PE involvement:
                            # DVE copies ctx out of PSUM (frees the bank), DVE
                            # reciprocal, GpSimd broadcasts 1/l over partitions.
                            ctxf = misc.tile([128, 1024], F32, tag="ctxf",
                                             name=f"cf{sb}{b}{h}")
                            nc.vector.tensor_copy(ctxf[:], ctx[:])
                            linv = misc.tile([1, 1024], F32, tag="linv",
                                             name=f"li{sb}{b}{h}")
                            nc.vector.reciprocal_approx_fast(out=linv[:], in_=lps[:])
                            bsb = misc.tile([128, 1024], F32, tag="bsb",
                                            name=f"bs{sb}{b}{h}")
                            nc.gpsimd.partition_broadcast(bsb[:], linv[:])
                            ctxn = misc.tile([128, 1024], BF, tag="ctxn",
                                             name=f"cn{sb}{b}{h}")
                            # on GpSimd (idle in B): keeps the DVE queue free
                            # of any dependence on the broadcast, so the next
                            # instance's ctxf/recip never head-of-line block.
                            nc.gpsimd.tensor_mul(ctxn[:], ctxf[:], bsb[:])
                            for tp in range(4):
                                d = 4 * b + tp
                                r0 = 256 * d + 128 * h
                                nc.sync.dma_start(
                                    a2a_in[sb][r0:r0 + 128, :],
                                    ctxn[:, 256 * tp:256 * tp + 256],
                                )
                    if sb == 0:
                        nc.sync.dma_start(
                            wo_sb[:].rearrange("ki ko c -> ki (ko c)"),
                            wo.ap())
                    nc.gpsimd.collective_compute(
                        "AllToAll", mybir.AluOpType.bypass,
                        ins=[a2a_in[sb][:]], outs=[a2a_out[sb][:]],
                        replica_groups=[[0, 1, 2, 3, 4, 5, 6, 7]],
                    )
                    for ko in range(16):
                        nc.sync.dma_start(
                            ctxg[sb][:, ko, :],
                            a2a_out[sb][128 * ko:128 * ko + 128, :],
                        )

            # ---- phase C: output projection on own s rows -----------------
            osb_pool = bc.enter_context(tc.tile_pool(name="osbp", bufs=2))
            ps_o = bc.enter_context(tc.tile_pool(name="pso", bufs=2, space="PSUM"))
            for half in range(2):
                for mm in range(2):
                    po = [ps_o.tile([128, 512], F32, tag=f"po{cc}",
                                    name=f"po{half}{mm}_{cc}") for cc in range(4)]
                    for ko in range(16):
                        for cc in range(4):
                            nc.tensor.matmul(
                                po[cc][:], ctxg[half][:, ko, 128 * mm:128 * mm + 128],
                                wo_sb[:, ko, 512 * cc:512 * cc + 512],
                                start=(ko == 0), stop=(ko == 15),
                                skip_group_check=True,
                            )
                    osb = osb_pool.tile([128, D], F32, tag="osb",
                                        name=f"osb{half}{mm}")
                    for cc in range(4):
                        nc.scalar.copy(osb[:, 512 * cc:512 * cc + 512], po[cc][:])
                    r0 = 256 * half + 128 * mm
                    nc.sync.dma_start(out.ap()[r0:r0 + 128, :], osb[:])

    nc.compile()
    return nc


def _get_runner():
    """Build (once) a persistent jitted SPMD executor for the kernel program."""
    if "runner" in _STATE:
        return _STATE["runner"]
    import jax
    from jax.sharding import Mesh, PartitionSpec
    from jax.experimental.shard_map import shard_map
    from concourse import bass2jax

    nc = _build()
    bass2jax.install_neuronx_cc_hook()

    in_names, out_names, out_avals = [], [], []
    for alloc in nc.m.functions[0].allocations:
        if not isinstance(alloc, mybir.MemoryLocationSet):
            continue
        name = alloc.memorylocations[0].name
        pname = nc.partition_id_tensor.name if nc.partition_id_tensor else None
        if alloc.kind == "ExternalInput":
            if name != pname:
                in_names.append(name)
        elif alloc.kind == "ExternalOutput":
            out_names.append(name)
            out_avals.append(
                jax.core.ShapedArray(tuple(alloc.tensor_shape),
                                     mybir.dt.np(alloc.dtype))
            )
    n_params = len(in_names)
    all_in = list(in_names) + list(out_names)
    pname = nc.partition_id_tensor.name if nc.partition_id_tensor else None
    if pname is not None:
        all_in.append(pname)

    def _body(*args):
        operands = list(args)
        if pname is not None:
            operands.append(bass2jax.partition_id_tensor())
        outs = bass2jax._bass_exec_p.bind(
            *operands,
            out_avals=tuple(out_avals),
            in_names=tuple(all_in),
            out_names=tuple(out_names),
            lowering_input_output_aliases=(),
            sim_require_finite=False,
            sim_require_nnan=False,
            nc=nc,
        )
        return tuple(outs)

    devices = jax.devices()[:NCORES]
    mesh = Mesh(np.asarray(devices), ("core",))
    specs = (PartitionSpec("core"),)
    sharded = jax.jit(
        shard_map(
            _body, mesh=mesh,
            in_specs=specs * (n_params + len(out_names)),
            out_specs=specs * len(out_names),
            check_rep=False,
        ),
        keep_unused=True,
    )
    runner = {
        "fn": sharded, "in_names": in_names, "out_names": out_names,
        "out_avals": out_avals, "n_params": n_params, "nc": nc,
    }
    _STATE["runner"] = runner
    return runner


def _prep_inputs(x, cos, sin, w_qkv, w_o):
    """Host-side sharding: per-core input dict list."""
    bf = ml_dtypes.bfloat16
    x = np.asarray(x, dtype=np.float32)
    cos = np.asarray(cos, dtype=np.float32)
    sin = np.asarray(sin, dtype=np.float32)
    w_qkv = np.asarray(w_qkv, dtype=np.float32)
    w_o = np.asarray(w_o, dtype=np.float32)

    cosT = np.ascontiguousarray(cos.T)                      # [128, S]
    sinT = sin.T
    sinTs = np.concatenate([-sinT[0:64], sinT[64:128]], axis=0)
    cosq = cosT.astype(bf)
    sinq = sinTs.astype(bf)
    cosk = (cosT * SCALE).astype(bf)
    sink = (sinTs * SCALE).astype(bf)
    pp, ff = np.meshgrid(np.arange(128), np.arange(128), indexing="ij")
    trimask = (pp <= ff).astype(bf)                         # keep t <= s

    def _swz_x(xT, chunks):
        """[D, S] -> [128, 16*S]: concat over chunks of [ki, ko, w] blocks."""
        xr = xT.reshape(16, 128, S)                    # [ko, ki, s]
        parts = [np.transpose(xr[:, :, s0:s1], (1, 0, 2)).reshape(128, -1)
                 for (s0, s1) in chunks]
        return np.ascontiguousarray(np.concatenate(parts, axis=1))

    def _swz_w(w, blocks):
        """[D, C] -> [128, 16*C]: concat over col blocks of [ki, ko, w]."""
        wr = w.reshape(16, 128, w.shape[1])            # [ko, ki, c]
        parts = [np.transpose(wr[:, :, c0:c1], (1, 0, 2)).reshape(128, -1)
                 for (c0, c1) in blocks]
        return np.ascontiguousarray(np.concatenate(parts, axis=1))

    xT0 = _swz_x(x[0].T.astype(bf), CH0)
    xT1 = _swz_x(x[1].T.astype(bf), CH1)
    wo_bf = _swz_w(w_o.astype(bf), [(0, D)])

    in_maps = []
    for c in range(NCORES):
        c0 = 128 * HLOC * c
        wq = w_qkv[:, c0:c0 + 256]
        wk = w_qkv[:, D + c0:D + c0 + 256]
        wv = w_qkv[:, 2 * D + c0:2 * D + c0 + 256]
        wqkv_c = _swz_w(
            np.concatenate([wq, wk, wv], axis=1).astype(bf), WBLK)
        in_maps.append({
            "xT0": xT0, "xT1": xT1, "wqkv": wqkv_c, "wo": wo_bf,
            "cosq": cosq, "sinq": sinq, "cosk": cosk, "sink": sink,
            "trimask": trimask,
        })
    return in_maps


def _run(in_maps):
    import jax
    r = _get_runner()
    concat = [
        np.concatenate([np.asarray(in_maps[c][n]) for c in range(NCORES)], axis=0)
        for n in r["in_names"]
    ]
    zeros = [
        np.zeros((NCORES * a.shape[0],) + tuple(a.shape[1:]), a.dtype)
        for a in r["out_avals"]
    ]
    outs = r["fn"](*concat, *zeros)
    outs = [np.asarray(o) for o in jax.block_until_ready(outs)]
    per_core = []
    for c in range(NCORES):
        d = {}
        for i, n in enumerate(r["out_names"]):
            shp = r["out_avals"][i].shape
            d[n] = outs[i].reshape((NCORES,) + shp)[c]
        per_core.append(d)
    return per_core


def kernel(x, cos, sin, w_qkv, w_o):
    in_maps = _prep_inputs(x, cos, sin, w_qkv, w_o)
    results = _run(in_maps)
    B = x.shape[0]
    out = np.empty((B, S, D), dtype=np.float32)
    for c in range(NCORES):
        g, tp = c // 4, c % 4
        res = results[c]["out"]
        out[g, 256 * tp:256 * tp + 256, :] = res[0:256]
        out[g, 1024 + 256 * tp:1024 + 256 * tp + 256, :] = res[256:512]
    return out
